# revision 12
# baseline (speedup 1.0000x reference)
"""Bass/Trainium2 kernel for nn_CrossAttention_57964878627478.

Reference computation (per batch b, per direction):
    q = Wq @ src + bq            [32, 4096]   (src = x for dir 0, y for dir 1)
    k = Wk @ ctx + bk            [32, 4096]   (ctx = the other tensor)
    v = Wv @ ctx + bv            [256, 4096]
    attn = softmax_j(q^T k)      [4096, 4096]
    out  = v @ attn^T            [256, 4096]

Sharding: 8 independent (batch, direction) pairs -> one per NeuronCore.

Per-core kernel layout choices:
  * S^T = k^T q computed in [j, i] layout directly (no transposes needed
    anywhere): lhsT = k strip [K=32, M=128 j], rhs = q strip [K=32, N=512 i].
    K=32 matmuls are packed 4-way with PE row tiling (tile_position) using
    4x-replicated q/k (replication is free: host tiles Wq^T/Wk^T columns).
  * exp on ScalarE, PSUM->SBUF, bf16 out, constant bias shift (softmax is
    shift-invariant; global max score ~34 fits fp32/bf16 range comfortably).
  * out^T[i, c] = P^T.T @ v^T via lhsT = P^T tile, rhs = v^T. An extra ones
    column appended to v^T makes column 256 of the PSUM accumulator the
    softmax denominator (free). Normalization is then a native per-partition
    tensor_scalar multiply. v carries its bias (folded in via a K=1 matmul
    with a ones lhsT), so out = psum[:, :256] * (1/psum[:, 256]) exactly.
  * q/k in fp16 (3 extra mantissa bits vs bf16 -> ~4x better end-to-end
    error), P/v in bf16 (P needs bf16's exponent range: fp16 underflows).
  * Output written as out^T [4096, 256] fp32; host transposes.

Schedule (keeps the PE continuously busy so the p-state ramp reaches and
holds the 2.4 GHz clock):
  * Input DMAs spread across the three HWDGE rings (SP/ACT/DVE queues),
    weights+biases first so the first projection can start ~2us in.
  * Phase C interleaves q/k projections, the t=0 S^T groups, and the v
    projection on the PE; the q/k PSUM->SBUF bias-copies alternate between
    DVE (tensor_scalar_add) and ACT (activation Identity with bias) so
    neither trailing engine gates the PE.
  * v-projection PSUM tiles share the out-accumulator pool ring ("po" tag):
    PSUM stays within 8 banks (3x2 st + 2x1 po).
"""

import sys

if "/opt/trn_rl_repo" not in sys.path:
    sys.path.insert(0, "/opt/trn_rl_repo")

import numpy as np
import ml_dtypes

C = 256
CQ = 32
HW = 4096
B = 4
N_CORES = 8
EXP_BIAS = -12.0

_cache = {}


def _build_program():
    from contextlib import ExitStack

    import concourse.bacc as bacc
    import concourse.mybir as mybir
    import concourse.tile as tile

    fp16 = mybir.dt.float16
    bf16 = mybir.dt.bfloat16
    f32 = mybir.dt.float32

    nc = bacc.Bacc(None, target_bir_lowering=False, debug=False)
    SRC = nc.dram_tensor("src", [C, HW], fp16, kind="ExternalInput")
    CTX = nc.dram_tensor("ctx", [C, HW], fp16, kind="ExternalInput")
    WQT = nc.dram_tensor("wqt", [C, 128], fp16, kind="ExternalInput")
    WKT = nc.dram_tensor("wkt", [C, 128], fp16, kind="ExternalInput")
    WVT = nc.dram_tensor("wvt", [C, C], fp16, kind="ExternalInput")
    BQ = nc.dram_tensor("bq_rep", [128, 1], f32, kind="ExternalInput")
    BK = nc.dram_tensor("bk_rep", [128, 1], f32, kind="ExternalInput")
    BV = nc.dram_tensor("bv_row", [1, C], f32, kind="ExternalInput")
    OUT = nc.dram_tensor("out_t", [HW, C], f32, kind="ExternalOutput")

    Exp = mybir.ActivationFunctionType.Exp
    Identity = mybir.ActivationFunctionType.Identity

    import os
    ST_GROUP = int(os.environ.get("K_ST_GROUP", "2"))   # j chunks per S^T psum tile
    ST_BUFS = int(os.environ.get("K_ST_BUFS", "3"))
    N_GROUPS = 32 // ST_GROUP                           # S^T groups per i tile
    P_BUFS = 2 * N_GROUPS

    with tile.TileContext(nc) as tc, ExitStack() as ctx:
        consts = ctx.enter_context(tc.tile_pool(name="consts", bufs=1))
        # PSUM budget (8 banks): st ST_BUFS x [128, 512*ST_GROUP] (also used
        # by the q/k projections) + po 2x[128,257] (also used by the v
        # projection in phase C) = 3*2 + 2 = 8.
        ps_st = ctx.enter_context(tc.tile_pool(name="ps_st", bufs=ST_BUFS, space="PSUM"))
        ps_out = ctx.enter_context(tc.tile_pool(name="ps_out", bufs=2, space="PSUM"))
        p_pool = ctx.enter_context(tc.tile_pool(name="p_pool", bufs=P_BUFS))
        o_pool = ctx.enter_context(tc.tile_pool(name="o_pool", bufs=4))
        r_pool = ctx.enter_context(tc.tile_pool(name="r_pool", bufs=4))

        # ---- constant / input staging ----
        # Weights + biases first (they gate the first projection matmul),
        # spread across the SP/ACT/DVE HWDGE rings; then the src/ctx column
        # chunks round-robin so chunk n lands ~in step with its projection.
        wqt_sb = consts.tile([128, 2, 128], fp16, tag="wqt_sb")
        wkt_sb = consts.tile([128, 2, 128], fp16, tag="wkt_sb")
        wvt_sb = consts.tile([128, 2, C], fp16, tag="wvt_sb")
        bq_sb = consts.tile([128, 1], f32, tag="bq_sb")
        bk_sb = consts.tile([128, 1], f32, tag="bk_sb")
        bv_bc = consts.tile([128, C], f32, tag="bv_bc")
        src_sb = consts.tile([128, 2, HW], fp16, tag="src_sb")
        ctx_sb = consts.tile([128, 2, HW], fp16, tag="ctx_sb")
        src_r = SRC[:].rearrange("(c p) j -> p c j", p=128)
        ctx_r = CTX[:].rearrange("(c p) j -> p c j", p=128)
        # Ring layout tuned for the critical chain wqt+src0 -> first q-proj
        # matmul: src0 heads SP's ring while wqt heads ACT's (both land
        # ~2.5us); ctx0/wkt right behind gate the first k-proj. ACT's SEQ is
        # free until the first exp (~6us), so it also carries chunks 1-2;
        # gpsimd's SWDGE carries the v-projection constants (first needed
        # ~4us) and one mid-stream chunk pair.
        def chunk(tile_r, n):
            return tile_r[:, :, n * 512:(n + 1) * 512]

        # The cost model's HWDGE and DMA transfer stages are each a single
        # serial device, so what matters is that the global processing order
        # matches the PE's consumption order. The smalls (biases, wvt, bv)
        # ride gpsimd's SWDGE which bypasses the HWDGE device entirely.
        # Chunk 0 is split into c-halves: the first q-proj matmul only needs
        # the c=0 half of src0 and wqt, so it starts one transfer earlier.
        nc.sync.dma_start(out=src_sb[:, 0, 0:512], in_=src_r[:, 0, 0:512])
        nc.scalar.dma_start(out=wqt_sb, in_=WQT[:].rearrange("(c p) m -> p c m", p=128))
        nc.sync.dma_start(out=src_sb[:, 1, 0:512], in_=src_r[:, 1, 0:512])
        nc.scalar.dma_start(out=ctx_sb[:, 0, 0:512], in_=ctx_r[:, 0, 0:512])
        nc.sync.dma_start(out=wkt_sb, in_=WKT[:].rearrange("(c p) m -> p c m", p=128))
        nc.scalar.dma_start(out=ctx_sb[:, 1, 0:512], in_=ctx_r[:, 1, 0:512])
        nc.gpsimd.dma_start(out=bq_sb, in_=BQ[:])
        nc.gpsimd.dma_start(out=bk_sb, in_=BK[:])
        nc.gpsimd.dma_start(out=wvt_sb, in_=WVT[:].rearrange("(c p) m -> p c m", p=128))
        nc.gpsimd.dma_start(out=bv_bc, in_=BV[:].to_broadcast((128, C)))
        for n in range(1, 8):
            eng = {2: nc.scalar, 6: nc.gpsimd}.get(n, nc.sync)
            eng.dma_start(out=chunk(src_sb, n), in_=chunk(src_r, n))
            eng.dma_start(out=chunk(ctx_sb, n), in_=chunk(ctx_r, n))

        ebias_sb = consts.tile([128, 1], f32, tag="ebias_sb")
        nc.vector.memset(ebias_sb, EXP_BIAS)

        q_sb = consts.tile([128, HW], fp16, tag="q_sb")
        k_sb = consts.tile([128, HW], fp16, tag="k_sb")
        vT_sb = consts.tile([128, 32, 257], bf16, tag="vT_sb")
        nc.vector.memset(vT_sb[:, :, 256], 1.0)

        # ---- projection helpers ----
        # q_rep/k_rep [128, HW]: Wq^T tiled 4x along columns by the host, so
        # the 4 partition strips hold identical copies of q (for row tiling).
        def qk_proj(n, wt, inp, bias, out_sb):
            ns = slice(n * 512, (n + 1) * 512)
            ps = ps_st.tile([128, 512], f32, tag="st")
            nc.tensor.matmul(ps, lhsT=wt[:, 0, :], rhs=inp[:, 0, ns],
                             start=True, stop=False)
            nc.tensor.matmul(ps, lhsT=wt[:, 1, :], rhs=inp[:, 1, ns],
                             start=False, stop=True)
            nc.vector.tensor_scalar_add(out_sb[:, ns], ps, bias)

        # v^T [j, c] per 128-row j chunk; bias added during the PSUM->SBUF
        # move (broadcast bv tile), alternating DVE/Pool so neither trailing
        # engine gates the PE in phase C. PSUM from the "po" ring.
        def v_proj(jc):
            js = slice(jc * 128, (jc + 1) * 128)
            psv = ps_out.tile([128, 257], f32, tag="po")
            nc.tensor.matmul(psv[:, 0:256], lhsT=ctx_sb[:, 0, js],
                             rhs=wvt_sb[:, 0, :], start=True, stop=False)
            nc.tensor.matmul(psv[:, 0:256], lhsT=ctx_sb[:, 1, js],
                             rhs=wvt_sb[:, 1, :], start=False, stop=True)
            nc.vector.tensor_add(vT_sb[:, jc, 0:256], psv[:, 0:256], bv_bc)

        # ---- attention building block ----
        def st_group(t, g):
            """S^T + exp for j chunks [g*ST_GROUP, (g+1)*ST_GROUP) of i-tile t."""
            isl = slice(t * 512, (t + 1) * 512)
            ps = ps_st.tile([128, 512 * ST_GROUP], f32, tag="st")
            for s in range(ST_GROUP):
                jc = g * ST_GROUP + s
                strip = jc % 4
                pb = slice(32 * strip, 32 * strip + 32)
                nc.tensor.matmul(
                    ps[:, s * 512:(s + 1) * 512],
                    lhsT=k_sb[pb, jc * 128:(jc + 1) * 128],
                    rhs=q_sb[pb, isl],
                    start=True, stop=True,
                    tile_position=(32 * strip, 0),
                )
            pt = p_pool.tile([128, 512 * ST_GROUP], bf16, tag="P")
            nc.scalar.activation(pt, ps, Exp, bias=ebias_sb)
            return pt

        def p_slice(p_tiles, jc, u):
            pt = p_tiles[jc // ST_GROUP]
            off = (jc % ST_GROUP) * 512 + u * 128
            return pt[:, off:off + 128]

        # ---- phase C: q/k proj + t=0 S^T groups + v proj, interleaved ----
        # st_group(0, 2m..2m+1) needs k chunk m (and its bias-copy), so lag
        # the S^T groups one chunk behind the k projection; v chunks fill the
        # remaining PE slots so ACT/DVE latency never throttles the PE.
        # q-projections for chunks 2-7 are deferred into the main loop (q
        # chunk t is first read by tile t): phase C's DVE load (k copies + v
        # bias-adds) is already at the PE's pace.
        p_cur = []
        for n in range(8):
            if n < 2:
                qk_proj(n, wqt_sb, src_sb, bq_sb, q_sb)
            qk_proj(n, wkt_sb, ctx_sb, bk_sb, k_sb)
            if n >= 1:
                m = n - 1
                for jc in range(4 * m, 4 * m + 4):
                    v_proj(jc)
                for g in range(2 * m, 2 * m + 2):
                    p_cur.append(st_group(0, g))
        for jc in range(28, 32):
            v_proj(jc)
        for g in range(14, 16):
            p_cur.append(st_group(0, g))

        # ---- main loop, software-pipelined ----
        # S^T/exp for i-tile t+1 is emitted interleaved with the out-matmuls
        # of i-tile t, so the single S^T PSUM buffer never stalls the PE:
        # between two quads the PE always has out-matmul work, and the exp
        # of a quad runs on ACT in that shadow.
        for t in range(8):  # i tiles of 512 query positions
            if t + 2 < 8:
                qk_proj(t + 2, wqt_sb, src_sb, bq_sb, q_sb)
            p_next = []
            for u in range(4):  # 128-row output chunks within the i tile
                gpu = N_GROUPS // 4  # groups to emit per u
                if t + 1 < 8:
                    for g in range(gpu * u, gpu * u + (gpu + 1) // 2):
                        p_next.append(st_group(t + 1, g))
                po = ps_out.tile([128, 257], f32, tag="po")
                for jc in range(16):
                    nc.tensor.matmul(po, lhsT=p_slice(p_cur, jc, u),
                                     rhs=vT_sb[:, jc, :],
                                     start=(jc == 0), stop=False)
                if t + 1 < 8:
                    for g in range(gpu * u + (gpu + 1) // 2, gpu * (u + 1)):
                        p_next.append(st_group(t + 1, g))
                for jc in range(16, 32):
                    nc.tensor.matmul(po, lhsT=p_slice(p_cur, jc, u),
                                     rhs=vT_sb[:, jc, :],
                                     start=False, stop=(jc == 31))
                rec = r_pool.tile([128, 1], f32, tag="rec")
                nc.vector.reciprocal(rec, po[:, 256:257])
                osb = o_pool.tile([128, 256], f32, tag="osb")
                nc.vector.tensor_scalar_mul(osb, po[:, 0:256], rec)
                row = t * 512 + u * 128
                nc.sync.dma_start(out=OUT[row:row + 128, :], in_=osb)
            p_cur = p_next

    nc.finalize()
    return nc


def _prep_shared(Wq, bq, Wk, bk, Wv, bv):
    wqt = np.tile(np.ascontiguousarray(Wq.T), (1, 4)).astype(np.float16)
    wkt = np.tile(np.ascontiguousarray(Wk.T), (1, 4)).astype(np.float16)
    wvt = np.ascontiguousarray(Wv.T).astype(np.float16)
    bq_rep = np.tile(bq.astype(np.float32), 4)[:, None]
    bk_rep = np.tile(bk.astype(np.float32), 4)[:, None]
    bv_row = bv.astype(np.float32)[None, :]
    return {
        "wqt": wqt, "wkt": wkt, "wvt": wvt,
        "bq_rep": np.ascontiguousarray(bq_rep),
        "bk_rep": np.ascontiguousarray(bk_rep),
        "bv_row": np.ascontiguousarray(bv_row.astype(np.float32)),
    }


def kernel(x, y, Wq, bq, Wk, bk, Wv, bv):
    from concourse.bass_utils import run_bass_kernel_spmd

    if "nc" not in _cache:
        _cache["nc"] = _build_program()
    nc = _cache["nc"]

    shared = _prep_shared(Wq, bq, Wk, bk, Wv, bv)
    x2 = np.asarray(x, dtype=np.float32).reshape(B, C, HW)
    y2 = np.asarray(y, dtype=np.float32).reshape(B, C, HW)

    in_maps = []
    for core in range(N_CORES):
        d, b = divmod(core, B)
        src = x2[b] if d == 0 else y2[b]
        ctxm = y2[b] if d == 0 else x2[b]
        m = dict(shared)
        m["src"] = np.ascontiguousarray(src.astype(np.float16))
        m["ctx"] = np.ascontiguousarray(ctxm.astype(np.float16))
        in_maps.append(m)

    res = run_bass_kernel_spmd(nc, in_maps, list(range(N_CORES)))
    outs = [r["out_t"] for r in res.results]  # each [HW, C] fp32, transposed

    outx = np.stack([np.ascontiguousarray(outs[b].T).reshape(C, 64, 64)
                     for b in range(B)])
    outy = np.stack([np.ascontiguousarray(outs[B + b].T).reshape(C, 64, 64)
                     for b in range(B)])
    return (outx.astype(np.float32), outy.astype(np.float32))


# revision 15
# speedup vs baseline: 1.0022x; 1.0022x over previous
"""Bass/Trainium2 kernel for nn_CrossAttention_57964878627478.

Reference computation (per batch b, per direction):
    q = Wq @ src + bq            [32, 4096]   (src = x for dir 0, y for dir 1)
    k = Wk @ ctx + bk            [32, 4096]   (ctx = the other tensor)
    v = Wv @ ctx + bv            [256, 4096]
    attn = softmax_j(q^T k)      [4096, 4096]
    out  = v @ attn^T            [256, 4096]

Sharding: 8 independent (batch, direction) pairs -> one per NeuronCore.

Per-core kernel layout choices:
  * S^T = k^T q computed in [j, i] layout directly (no transposes needed
    anywhere): lhsT = k strip [K=32, M=128 j], rhs = q strip [K=32, N=512 i].
    K=32 matmuls are packed 4-way with PE row tiling (tile_position) using
    4x-replicated q/k (replication is free: host tiles Wq^T/Wk^T columns).
  * exp on ScalarE, PSUM->SBUF, bf16 out, constant bias shift (softmax is
    shift-invariant; global max score ~34 fits fp32/bf16 range comfortably).
  * out^T[i, c] = P^T.T @ v^T via lhsT = P^T tile, rhs = v^T. An extra ones
    column appended to v^T makes column 256 of the PSUM accumulator the
    softmax denominator (free). Normalization is then a native per-partition
    tensor_scalar multiply. v carries its bias (folded in via a K=1 matmul
    with a ones lhsT), so out = psum[:, :256] * (1/psum[:, 256]) exactly.
  * q/k in fp16 (3 extra mantissa bits vs bf16 -> ~4x better end-to-end
    error), P/v in bf16 (P needs bf16's exponent range: fp16 underflows).
  * Output written as out^T [4096, 256] fp32; host transposes.

Schedule (keeps the PE continuously busy so the p-state ramp reaches and
holds the 2.4 GHz clock):
  * Input DMAs spread across the three HWDGE rings (SP/ACT/DVE queues),
    weights+biases first so the first projection can start ~2us in.
  * Phase C interleaves q/k projections, the t=0 S^T groups, and the v
    projection on the PE; the q/k PSUM->SBUF bias-copies alternate between
    DVE (tensor_scalar_add) and ACT (activation Identity with bias) so
    neither trailing engine gates the PE.
  * v-projection PSUM tiles share the out-accumulator pool ring ("po" tag):
    PSUM stays within 8 banks (3x2 st + 2x1 po).
"""

import sys

if "/opt/trn_rl_repo" not in sys.path:
    sys.path.insert(0, "/opt/trn_rl_repo")

import numpy as np
import ml_dtypes

C = 256
CQ = 32
HW = 4096
B = 4
N_CORES = 8
EXP_BIAS = -12.0

_cache = {}


def _build_program():
    from contextlib import ExitStack

    import concourse.bacc as bacc
    import concourse.mybir as mybir
    import concourse.tile as tile

    fp16 = mybir.dt.float16
    bf16 = mybir.dt.bfloat16
    f32 = mybir.dt.float32

    nc = bacc.Bacc(None, target_bir_lowering=False, debug=False)
    SRC = nc.dram_tensor("src", [C, HW], fp16, kind="ExternalInput")
    CTX = nc.dram_tensor("ctx", [C, HW], fp16, kind="ExternalInput")
    WQT = nc.dram_tensor("wqt", [C, 128], fp16, kind="ExternalInput")
    WKT = nc.dram_tensor("wkt", [C, 128], fp16, kind="ExternalInput")
    WVT = nc.dram_tensor("wvt", [C, C], fp16, kind="ExternalInput")
    BQ = nc.dram_tensor("bq_rep", [128, 1], f32, kind="ExternalInput")
    BK = nc.dram_tensor("bk_rep", [128, 1], f32, kind="ExternalInput")
    BV = nc.dram_tensor("bv_row", [1, C], f32, kind="ExternalInput")
    OUT = nc.dram_tensor("out_t", [HW, C], f32, kind="ExternalOutput")

    Exp = mybir.ActivationFunctionType.Exp
    Identity = mybir.ActivationFunctionType.Identity

    import os
    ST_GROUP = int(os.environ.get("K_ST_GROUP", "2"))   # j chunks per S^T psum tile
    ST_BUFS = int(os.environ.get("K_ST_BUFS", "3"))
    N_GROUPS = 32 // ST_GROUP                           # S^T groups per i tile
    P_BUFS = 2 * N_GROUPS

    with tile.TileContext(nc) as tc, ExitStack() as ctx:
        consts = ctx.enter_context(tc.tile_pool(name="consts", bufs=1))
        # PSUM budget (8 banks): st ST_BUFS x [128, 512*ST_GROUP] (also used
        # by the q/k projections) + po 2x[128,257] (also used by the v
        # projection in phase C) = 3*2 + 2 = 8.
        ps_st = ctx.enter_context(tc.tile_pool(name="ps_st", bufs=ST_BUFS, space="PSUM"))
        ps_out = ctx.enter_context(tc.tile_pool(name="ps_out", bufs=2, space="PSUM"))
        p_pool = ctx.enter_context(tc.tile_pool(name="p_pool", bufs=P_BUFS))
        o_pool = ctx.enter_context(tc.tile_pool(name="o_pool", bufs=4))
        r_pool = ctx.enter_context(tc.tile_pool(name="r_pool", bufs=4))

        # ---- constant / input staging ----
        # Weights + biases first (they gate the first projection matmul),
        # spread across the SP/ACT/DVE HWDGE rings; then the src/ctx column
        # chunks round-robin so chunk n lands ~in step with its projection.
        wqt_sb = consts.tile([128, 2, 128], fp16, tag="wqt_sb")
        wkt_sb = consts.tile([128, 2, 128], fp16, tag="wkt_sb")
        wvt_sb = consts.tile([128, 2, C], fp16, tag="wvt_sb")
        bq_sb = consts.tile([128, 1], f32, tag="bq_sb")
        bk_sb = consts.tile([128, 1], f32, tag="bk_sb")
        bv_bc = consts.tile([128, C], f32, tag="bv_bc")
        src_sb = consts.tile([128, 2, HW], fp16, tag="src_sb")
        ctx_sb = consts.tile([128, 2, HW], fp16, tag="ctx_sb")
        src_r = SRC[:].rearrange("(c p) j -> p c j", p=128)
        ctx_r = CTX[:].rearrange("(c p) j -> p c j", p=128)
        # Ring layout tuned for the critical chain wqt+src0 -> first q-proj
        # matmul: src0 heads SP's ring while wqt heads ACT's (both land
        # ~2.5us); ctx0/wkt right behind gate the first k-proj. ACT's SEQ is
        # free until the first exp (~6us), so it also carries chunks 1-2;
        # gpsimd's SWDGE carries the v-projection constants (first needed
        # ~4us) and one mid-stream chunk pair.
        def chunk(tile_r, n):
            return tile_r[:, :, n * 512:(n + 1) * 512]

        # The cost model's HWDGE and DMA transfer stages are each a single
        # serial device, so what matters is that the global processing order
        # matches the PE's consumption order. The smalls (biases, wvt, bv)
        # ride gpsimd's SWDGE which bypasses the HWDGE device entirely.
        nc.sync.dma_start(out=chunk(src_sb, 0), in_=chunk(src_r, 0))
        nc.scalar.dma_start(out=wqt_sb, in_=WQT[:].rearrange("(c p) m -> p c m", p=128))
        nc.sync.dma_start(out=wkt_sb, in_=WKT[:].rearrange("(c p) m -> p c m", p=128))
        nc.scalar.dma_start(out=chunk(ctx_sb, 0), in_=chunk(ctx_r, 0))
        nc.gpsimd.dma_start(out=bq_sb, in_=BQ[:])
        nc.gpsimd.dma_start(out=bk_sb, in_=BK[:])
        nc.gpsimd.dma_start(out=wvt_sb, in_=WVT[:].rearrange("(c p) m -> p c m", p=128))
        nc.gpsimd.dma_start(out=bv_bc, in_=BV[:].to_broadcast((128, C)))
        for n in range(1, 8):
            eng = {2: nc.scalar, 6: nc.gpsimd}.get(n, nc.sync)
            eng.dma_start(out=chunk(src_sb, n), in_=chunk(src_r, n))
            eng.dma_start(out=chunk(ctx_sb, n), in_=chunk(ctx_r, n))

        ebias_sb = consts.tile([128, 1], f32, tag="ebias_sb")
        nc.vector.memset(ebias_sb, EXP_BIAS)

        q_sb = consts.tile([128, HW], fp16, tag="q_sb")
        k_sb = consts.tile([128, HW], fp16, tag="k_sb")
        vT_sb = consts.tile([128, 32, 257], bf16, tag="vT_sb")
        nc.vector.memset(vT_sb[:, :, 256], 1.0)

        # ---- projection helpers ----
        # q_rep/k_rep [128, HW]: Wq^T tiled 4x along columns by the host, so
        # the 4 partition strips hold identical copies of q (for row tiling).
        def qk_proj(n, wt, inp, bias, out_sb, halves=1):
            ns = slice(n * 512, (n + 1) * 512)
            ps = ps_st.tile([128, 512], f32, tag="st")
            nc.tensor.matmul(ps, lhsT=wt[:, 0, :], rhs=inp[:, 0, ns],
                             start=True, stop=False)
            nc.tensor.matmul(ps, lhsT=wt[:, 1, :], rhs=inp[:, 1, ns],
                             start=False, stop=True)
            # halves=2 halves the PSUM->SBUF copy latency: each S^T group
            # only reads a 256-column slice of k, so it unblocks sooner.
            for h in range(halves):
                hs = slice(h * (512 // halves), (h + 1) * (512 // halves))
                nc.vector.tensor_scalar_add(
                    out_sb[:, n * 512:(n + 1) * 512][:, hs], ps[:, hs], bias)

        # v^T [j, c] per 128-row j chunk; bias added during the PSUM->SBUF
        # move (broadcast bv tile), alternating DVE/Pool so neither trailing
        # engine gates the PE in phase C. PSUM from the "po" ring.
        def v_proj(jc):
            js = slice(jc * 128, (jc + 1) * 128)
            psv = ps_out.tile([128, 257], f32, tag="po")
            nc.tensor.matmul(psv[:, 0:256], lhsT=ctx_sb[:, 0, js],
                             rhs=wvt_sb[:, 0, :], start=True, stop=False)
            nc.tensor.matmul(psv[:, 0:256], lhsT=ctx_sb[:, 1, js],
                             rhs=wvt_sb[:, 1, :], start=False, stop=True)
            nc.vector.tensor_add(vT_sb[:, jc, 0:256], psv[:, 0:256], bv_bc)

        # ---- attention building block ----
        def st_group(t, g):
            """S^T + exp for j chunks [g*ST_GROUP, (g+1)*ST_GROUP) of i-tile t."""
            isl = slice(t * 512, (t + 1) * 512)
            ps = ps_st.tile([128, 512 * ST_GROUP], f32, tag="st")
            for s in range(ST_GROUP):
                jc = g * ST_GROUP + s
                strip = jc % 4
                pb = slice(32 * strip, 32 * strip + 32)
                nc.tensor.matmul(
                    ps[:, s * 512:(s + 1) * 512],
                    lhsT=k_sb[pb, jc * 128:(jc + 1) * 128],
                    rhs=q_sb[pb, isl],
                    start=True, stop=True,
                    tile_position=(32 * strip, 0),
                )
            pt = p_pool.tile([128, 512 * ST_GROUP], bf16, tag="P")
            nc.scalar.activation(pt, ps, Exp, bias=ebias_sb)
            return pt

        def p_slice(p_tiles, jc, u):
            pt = p_tiles[jc // ST_GROUP]
            off = (jc % ST_GROUP) * 512 + u * 128
            return pt[:, off:off + 128]

        # ---- phase C: q/k proj + t=0 S^T groups + v proj, interleaved ----
        # st_group(0, 2m..2m+1) needs k chunk m (and its bias-copy), so lag
        # the S^T groups one chunk behind the k projection; v chunks fill the
        # remaining PE slots so ACT/DVE latency never throttles the PE.
        # q-projections for chunks 2-7 are deferred into the main loop (q
        # chunk t is first read by tile t): phase C's DVE load (k copies + v
        # bias-adds) is already at the PE's pace.
        p_cur = []
        for n in range(8):
            if n < 2:
                qk_proj(n, wqt_sb, src_sb, bq_sb, q_sb)
            qk_proj(n, wkt_sb, ctx_sb, bk_sb, k_sb, halves=2)
            if n >= 1:
                m = n - 1
                for jc in range(4 * m, 4 * m + 4):
                    v_proj(jc)
                for g in range(2 * m, 2 * m + 2):
                    p_cur.append(st_group(0, g))
        for jc in range(28, 32):
            v_proj(jc)
        for g in range(14, 16):
            p_cur.append(st_group(0, g))

        # ---- main loop, software-pipelined ----
        # S^T/exp for i-tile t+1 is emitted interleaved with the out-matmuls
        # of i-tile t, so the single S^T PSUM buffer never stalls the PE:
        # between two quads the PE always has out-matmul work, and the exp
        # of a quad runs on ACT in that shadow.
        for t in range(8):  # i tiles of 512 query positions
            if t + 2 < 8:
                qk_proj(t + 2, wqt_sb, src_sb, bq_sb, q_sb)
            p_next = []
            for u in range(4):  # 128-row output chunks within the i tile
                gpu = N_GROUPS // 4  # groups to emit per u
                if t + 1 < 8:
                    for g in range(gpu * u, gpu * u + (gpu + 1) // 2):
                        p_next.append(st_group(t + 1, g))
                po = ps_out.tile([128, 257], f32, tag="po")
                for jc in range(16):
                    nc.tensor.matmul(po, lhsT=p_slice(p_cur, jc, u),
                                     rhs=vT_sb[:, jc, :],
                                     start=(jc == 0), stop=False)
                if t + 1 < 8:
                    for g in range(gpu * u + (gpu + 1) // 2, gpu * (u + 1)):
                        p_next.append(st_group(t + 1, g))
                for jc in range(16, 32):
                    nc.tensor.matmul(po, lhsT=p_slice(p_cur, jc, u),
                                     rhs=vT_sb[:, jc, :],
                                     start=False, stop=(jc == 31))
                rec = r_pool.tile([128, 1], f32, tag="rec")
                nc.vector.reciprocal(rec, po[:, 256:257])
                osb = o_pool.tile([128, 256], f32, tag="osb")
                nc.vector.tensor_scalar_mul(osb, po[:, 0:256], rec)
                row = t * 512 + u * 128
                nc.sync.dma_start(out=OUT[row:row + 128, :], in_=osb)
            p_cur = p_next

    nc.finalize()
    return nc


def _prep_shared(Wq, bq, Wk, bk, Wv, bv):
    wqt = np.tile(np.ascontiguousarray(Wq.T), (1, 4)).astype(np.float16)
    wkt = np.tile(np.ascontiguousarray(Wk.T), (1, 4)).astype(np.float16)
    wvt = np.ascontiguousarray(Wv.T).astype(np.float16)
    bq_rep = np.tile(bq.astype(np.float32), 4)[:, None]
    bk_rep = np.tile(bk.astype(np.float32), 4)[:, None]
    bv_row = bv.astype(np.float32)[None, :]
    return {
        "wqt": wqt, "wkt": wkt, "wvt": wvt,
        "bq_rep": np.ascontiguousarray(bq_rep),
        "bk_rep": np.ascontiguousarray(bk_rep),
        "bv_row": np.ascontiguousarray(bv_row.astype(np.float32)),
    }


def kernel(x, y, Wq, bq, Wk, bk, Wv, bv):
    from concourse.bass_utils import run_bass_kernel_spmd

    if "nc" not in _cache:
        _cache["nc"] = _build_program()
    nc = _cache["nc"]

    shared = _prep_shared(Wq, bq, Wk, bk, Wv, bv)
    x2 = np.asarray(x, dtype=np.float32).reshape(B, C, HW)
    y2 = np.asarray(y, dtype=np.float32).reshape(B, C, HW)

    in_maps = []
    for core in range(N_CORES):
        d, b = divmod(core, B)
        src = x2[b] if d == 0 else y2[b]
        ctxm = y2[b] if d == 0 else x2[b]
        m = dict(shared)
        m["src"] = np.ascontiguousarray(src.astype(np.float16))
        m["ctx"] = np.ascontiguousarray(ctxm.astype(np.float16))
        in_maps.append(m)

    res = run_bass_kernel_spmd(nc, in_maps, list(range(N_CORES)))
    outs = [r["out_t"] for r in res.results]  # each [HW, C] fp32, transposed

    outx = np.stack([np.ascontiguousarray(outs[b].T).reshape(C, 64, 64)
                     for b in range(B)])
    outy = np.stack([np.ascontiguousarray(outs[B + b].T).reshape(C, 64, 64)
                     for b in range(B)])
    return (outx.astype(np.float32), outy.astype(np.float32))
